# revision 24
# baseline (speedup 1.0000x reference)
"""Trainium2 Bass kernel for nn_DynamicReindexingRAG (B=4, N=1024, L=128, D=128, Q=64).

Math (equivalent to the reference):
- argsort+gather applies the SAME permutation to keys and docs; softmax-weighted
  sums are permutation invariant => no sorting needed.
- K = docs @ Wk.T + bk never materialized:
      s[b,m] = (Wk.T qp_sum) . docs[b,m] + const   (const cancels in softmax)
  Folding qp_sum = Q*(Wq qvec + bq):  c_t = A2 @ qvec_t + b2,
      A2 = Q * Wk.T @ Wq,   b2 = Q * Wk.T @ bq.
- Per step:  s = docs_flat @ c;  w = softmax(s);  out = w @ docs_flat;
  qvec' = 0.5*(qvec + out).

Implementation vs the 2.1ms baseline:
- docs stored in SBUF as fp16 for the WHOLE kernel (16 MiB/core fits in SBUF);
  HBM traffic drops from 32 MiB/step to 16 MiB once.
- scores: vector engine fp16 products + grouped reduce; exp on scalar engine;
  weighted sum: tensor engine fp16 chained matmuls (1 cycle/col vs 4 for fp32).
- cross-core softmax combine: one small AllGather per (batch, step), emitted so
  it overlaps the other batches' compute.
"""

import numpy as np

B, N, L, D, Q = 4, 1024, 128, 128, 64
NL = N * L
N_CORES = 8
MC = NL // N_CORES          # tokens per core per batch (16384)
NTILE = MC // 128           # 128 [128tok x 128d] tiles per batch
PAY = 2 + D                 # payload floats per batch: [M | S | O(128)]

_cache = {}


def build(max_steps: int, n_cores: int = N_CORES, cc_mode: str = "multi",
          skip: str = ""):
    import concourse.bass as bass
    import concourse.bacc as bacc
    import concourse.tile as tile
    import concourse.mybir as mybir
    from concourse import bass_isa
    from contextlib import ExitStack

    F32 = mybir.dt.float32
    F16 = mybir.dt.float16
    AF = mybir.ActivationFunctionType
    ALU = mybir.AluOpType
    AX = mybir.AxisListType

    nc = bacc.Bacc("TRN2", target_bir_lowering=False, debug=False,
                   num_devices=n_cores)
    docs_ap = nc.dram_tensor("docs", [128, B, NTILE, 128], F16,
                             kind="ExternalInput").ap()
    a2t_ap = nc.dram_tensor("a2t", [D, D], F32, kind="ExternalInput").ap()
    b2x_ap = nc.dram_tensor("b2x", [D, B], F32, kind="ExternalInput").ap()
    qv0_ap = nc.dram_tensor("qv0", [D, B], F32, kind="ExternalInput").ap()
    ident_ap = nc.dram_tensor("ident", [128, 128], F32, kind="ExternalInput").ap()
    outs_ap = nc.dram_tensor("outs", [max_steps * B, D], F32,
                             kind="ExternalOutput").ap()

    with tile.TileContext(nc) as tc, ExitStack() as ctx:
        const = ctx.enter_context(tc.tile_pool(name="const", bufs=1))
        docs_pool = ctx.enter_context(tc.tile_pool(name="docs", bufs=1))
        state = ctx.enter_context(tc.tile_pool(name="state", bufs=1))
        prod_pool = ctx.enter_context(tc.tile_pool(name="prod", bufs=1))
        tree_pool = ctx.enter_context(tc.tile_pool(name="tree", bufs=1))
        sbufs = ctx.enter_context(tc.tile_pool(name="sbufs", bufs=2))
        wpool = ctx.enter_context(tc.tile_pool(name="wpool", bufs=2))
        cbc_pool = ctx.enter_context(tc.tile_pool(name="cbc", bufs=3))
        small = ctx.enter_context(tc.tile_pool(name="small", bufs=6))
        mid = ctx.enter_context(tc.tile_pool(name="mid", bufs=2))
        pays = ctx.enter_context(tc.tile_pool(name="pays", bufs=3))
        gaths = ctx.enter_context(tc.tile_pool(name="gaths", bufs=3))
        ps_o = ctx.enter_context(tc.tile_pool(name="ps_o", bufs=2, space="PSUM"))
        ps_sm = ctx.enter_context(tc.tile_pool(name="ps_sm", bufs=3, space="PSUM"))
        ps_junk = ctx.enter_context(tc.tile_pool(name="ps_junk", bufs=1,
                                                 space="PSUM"))
        dram = ctx.enter_context(tc.tile_pool(name="dram", bufs=6, space="DRAM"))

        # ---- constants / state ----
        a2t = const.tile([D, D], F32)
        nc.sync.dma_start(a2t[:], a2t_ap[:])
        b2x = const.tile([D, B], F32)
        nc.sync.dma_start(b2x[:], b2x_ap[:])
        ident = const.tile([128, 128], F32)
        nc.sync.dma_start(ident[:], ident_ap[:])
        ones_col = const.tile([128, 1], F32)
        nc.vector.memset(ones_col[:], 1.0)
        junk_sb = const.tile([128, 128], F32)
        nc.vector.memset(junk_sb[:], 1.0)

        qv = state.tile([D, B], F32)
        nc.sync.dma_start(qv[:], qv0_ap[:])
        results = state.tile([D, max_steps * B], F32)

        docs_sb = []
        for b in range(B):
            dt_ = docs_pool.tile([128, NTILE, 128], F16, tag=f"docs{b}")
            nc.sync.dma_start(dt_[:], docs_ap[:, b])
            docs_sb.append(dt_)

        def build_c(b):
            # c_b = A2 @ qv[:, b] + b2, broadcast to all partitions, fp16
            c_ps = ps_sm.tile([D, 1], F32, tag="sm")
            nc.tensor.matmul(c_ps[:], a2t[:], qv[:, b:b + 1],
                             start=True, stop=True)
            c_sb = small.tile([D, 1], F32, tag="c_sb")
            nc.vector.tensor_tensor(c_sb[:], c_ps[:], b2x[:, b:b + 1],
                                    op=ALU.add)
            cT_ps = ps_sm.tile([1, D], F32, tag="sm")
            nc.tensor.transpose(cT_ps[:], c_sb[:], ident[:])
            cT16 = mid.tile([1, D], F16, tag="cT16")
            nc.scalar.copy(cT16[:], cT_ps[:])
            c_bc = cbc_pool.tile([128, D], F16, tag="c_bc")
            nc.gpsimd.partition_broadcast(c_bc[:], cT16[:])
            return c_bc

        junk_ps = ps_junk.tile([1, 128], F32, tag="junk")

        def front_half(t, b, c_bc, payload):
            """pass1 + softmax + pass2; writes [M|S|O] into payload [1, PAY]."""
            # keep-warm: independent junk matmuls keep the PE out of its slow
            # p-state while the vector engine computes this batch's scores.
            for _ in range(20):
                nc.tensor.matmul(junk_ps[:], ones_col[:], junk_sb[:],
                                 start=True, stop=True)
            d3 = docs_sb[b][:]                            # [128, NTILE, 128]
            s_buf = sbufs.tile([128, NTILE], F32, tag="s")
            if skip == "pass1":
                nc.vector.tensor_copy(s_buf[:], ident[:, 0:NTILE])
            else:
                c3 = c_bc[:].rearrange("p (o d) -> p o d", o=1)
                prod = prod_pool.tile([128, NTILE, 128], F16, tag="prod")
                i0, i1 = bass.broadcast_tensor_aps(d3, c3)
                nc.vector.tensor_tensor(prod[:], i0, i1, op=ALU.mult)
                if skip == "tree":
                    nc.vector.tensor_copy(s_buf[:], ident[:, 0:NTILE])
                else:
                    # pairwise f16 tree (tensor_tensor add runs at 2x; the 1x
                    # tensor_reduce would cost 16384 cycles alone). Ping-pong
                    # between a scratch tile and the (consumed) prod buffer.
                    tscr = tree_pool.tile([128, NTILE, 64], F16, tag="tree")
                    nc.vector.tensor_tensor(tscr[:], prod[:, :, 0:64],
                                            prod[:, :, 64:128], op=ALU.add)
                    nc.vector.tensor_tensor(prod[:, :, 0:32], tscr[:, :, 0:32],
                                            tscr[:, :, 32:64], op=ALU.add)
                    nc.vector.tensor_tensor(tscr[:, :, 0:16], prod[:, :, 0:16],
                                            prod[:, :, 16:32], op=ALU.add)
                    nc.vector.tensor_tensor(prod[:, :, 0:8], tscr[:, :, 0:8],
                                            tscr[:, :, 8:16], op=ALU.add)
                    nc.vector.tensor_tensor(tscr[:, :, 0:4], prod[:, :, 0:4],
                                            prod[:, :, 4:8], op=ALU.add)
                    nc.vector.tensor_tensor(prod[:, :, 0:2], tscr[:, :, 0:2],
                                            tscr[:, :, 2:4], op=ALU.add)
                    s3 = s_buf[:].rearrange("p (t o) -> p t o", o=1)
                    nc.vector.tensor_tensor(s3, prod[:, :, 0:1],
                                            prod[:, :, 1:2], op=ALU.add)

            m1 = small.tile([128, 1], F32, tag="m1")
            nc.vector.tensor_reduce(m1[:], s_buf[:], axis=AX.X, op=ALU.max)
            mall = small.tile([128, 1], F32, tag="mall")
            nc.gpsimd.partition_all_reduce(mall[:], m1[:], channels=128,
                                           reduce_op=bass_isa.ReduceOp.max)
            negm = small.tile([128, 1], F32, tag="negm")
            nc.scalar.mul(negm[:], mall[:], -1.0)
            w16 = wpool.tile([128, NTILE], F16, tag="w")
            wsum = small.tile([128, 1], F32, tag="wsum")
            nc.scalar.activation(w16[:], s_buf[:], AF.Exp, bias=negm[:],
                                 scale=1.0, accum_out=wsum[:])
            S_ps = ps_sm.tile([1, 1], F32, tag="sm")
            nc.tensor.matmul(S_ps[:], wsum[:], ones_col[:],
                             start=True, stop=True)

            # pass2 4-wide: lhsT = 4 w-columns, rhs = 4-tile span [128, 512].
            # Diagonal blocks of the [4, 512] psum accumulate exactly
            # sum_j w_j docs_j; off-diagonal blocks are ignored.
            ngrp = NTILE // 4
            o4_ps = ps_o.tile([4, 4 * D], F32, tag="o")
            if skip == "pass2":
                nc.tensor.matmul(o4_ps[:], w16[:, 0:4],
                                 d3[:, 0:4, :].rearrange("p t d -> p (t d)"),
                                 start=True, stop=True)
            else:
                for g in range(ngrp):
                    rhs = d3[:, 4 * g:4 * g + 4, :].rearrange(
                        "p t d -> p (t d)")
                    nc.tensor.matmul(o4_ps[:], w16[:, 4 * g:4 * g + 4], rhs,
                                     start=(g == 0), stop=(g == ngrp - 1))
            s45 = mid.tile([4, 4 * D], F32, tag="s4")
            nc.scalar.copy(s45[:], o4_ps[:])
            # diagonal extraction: sum_i s45[i, i*D:(i+1)*D] via 4 selector
            # matmuls (identity columns) accumulating into one psum row.
            o_ps2 = ps_sm.tile([1, D], F32, tag="sm")
            for i in range(4):
                nc.tensor.matmul(o_ps2[:], ident[0:4, i:i + 1],
                                 s45[:, i * D:(i + 1) * D],
                                 start=(i == 0), stop=(i == 3))

            nc.vector.tensor_copy(payload[0:1, 0:1], mall[0:1, :])
            nc.scalar.copy(payload[0:1, 1:2], S_ps[:])
            nc.scalar.copy(payload[0:1, 2:2 + D], o_ps2[:])

        def launch_cc(payload, width):
            if cc_mode == "none":
                gath = gaths.tile([n_cores, width], F32, tag="gath")
                nc.gpsimd.partition_broadcast(gath[:], payload[:])
                return gath
            cc_in = dram.tile([1, width], F32, tag="cc_in")
            cc_out = dram.tile([n_cores, width], F32, tag="cc_out")
            nc.sync.dma_start(cc_in[:], payload[:])
            nc.gpsimd.collective_compute(
                "AllGather", mybir.AluOpType.bypass,
                replica_groups=[list(range(n_cores))],
                ins=[cc_in.opt()], outs=[cc_out.opt()])
            gath = gaths.tile([n_cores, width], F32, tag="gath")
            nc.sync.dma_start(gath[:], cc_out[:])
            return gath

        def back_half(t, b, gath, off=0):
            """combine partials, write result, update qv."""
            Mg = small.tile([n_cores, 1], F32, tag="Mg")
            nc.gpsimd.partition_all_reduce(Mg[:], gath[:, off:off + 1],
                                           channels=n_cores,
                                           reduce_op=bass_isa.ReduceOp.max)
            negMg = small.tile([n_cores, 1], F32, tag="negMg")
            nc.scalar.mul(negMg[:], Mg[:], -1.0)
            f = small.tile([n_cores, 1], F32, tag="f")
            nc.scalar.activation(f[:], gath[:, off:off + 1], AF.Exp,
                                 bias=negMg[:], scale=1.0)
            St_ps = ps_sm.tile([1, 1], F32, tag="sm")
            nc.tensor.matmul(St_ps[:], gath[:, off + 1:off + 2], f[:],
                             start=True, stop=True)
            rS = small.tile([1, 1], F32, tag="rS")
            nc.vector.reciprocal(rS[:], St_ps[:])
            rS128 = small.tile([D, 1], F32, tag="rS128")
            nc.gpsimd.partition_broadcast(rS128[:], rS[:])
            Ow_ps = ps_sm.tile([D, 1], F32, tag="sm")
            nc.tensor.matmul(Ow_ps[:], gath[:, off + 2:off + 2 + D], f[:],
                             start=True, stop=True)
            out_col = small.tile([D, 1], F32, tag="out_col")
            nc.scalar.mul(out_col[:], Ow_ps[:], rS128[:])
            nc.vector.tensor_copy(results[:, t * B + b:t * B + b + 1],
                                  out_col[:])
            nc.vector.tensor_scalar(qv[:, b:b + 1], qv[:, b:b + 1],
                                    out_col[:], 0.5,
                                    op0=ALU.add, op1=ALU.mult)

        # software-pipelined emission: combine of (b, t-1) goes right before
        # the heavy compute of (b, t) so the AllGather latency hides behind
        # the other batches' work.
        cbcs = [build_c(b) for b in range(B)]
        if cc_mode == "single":
            # one AllGather per step; batch payloads packed along free dim
            gath_prev = None
            for t in range(max_steps):
                payrow = pays.tile([1, B * PAY], F32, tag="pay")
                for b in range(B):
                    if gath_prev is not None:
                        back_half(t - 1, b, gath_prev, off=b * PAY)
                        cbcs[b] = build_c(b)
                    front_half(t, b, cbcs[b],
                               payrow[0:1, b * PAY:(b + 1) * PAY])
                gath_prev = launch_cc(payrow, B * PAY)
            for b in range(B):
                back_half(max_steps - 1, b, gath_prev, off=b * PAY)
        else:
            gaths_prev = [None] * B
            for t in range(max_steps):
                for b in range(B):
                    if gaths_prev[b] is not None:
                        back_half(t - 1, b, gaths_prev[b])
                        cbcs[b] = build_c(b)
                    payload = pays.tile([1, PAY], F32, tag="pay")
                    front_half(t, b, cbcs[b], payload)
                    gaths_prev[b] = launch_cc(payload, PAY)
            for b in range(B):
                back_half(max_steps - 1, b, gaths_prev[b])

        # ---- outputs: results [D, S*B] -> outs [S*B, D], 128 cols at a time
        n_out = max_steps * B
        for k0 in range(0, n_out, 128):
            nk = min(128, n_out - k0)
            res_ps = ps_junk.tile([nk, D], F32, tag="res")
            nc.tensor.transpose(res_ps[:], results[:, k0:k0 + nk], ident[:])
            res_T = mid.tile([nk, D], F32, tag="resT")
            nc.scalar.copy(res_T[:], res_ps[:])
            nc.sync.dma_start(outs_ap[k0:k0 + nk, :], res_T[:])

    nc.compile()
    return nc


def make_inputs(query, documents, Wq, bq, Wk, bk, n_cores: int = N_CORES):
    """Host-side preprocessing -> per-core input maps."""
    query = np.asarray(query, dtype=np.float32)
    documents = np.asarray(documents, dtype=np.float32)
    Wq64 = np.asarray(Wq, dtype=np.float64)
    bq64 = np.asarray(bq, dtype=np.float64)
    Wk64 = np.asarray(Wk, dtype=np.float64)

    A2 = Q * (Wk64.T @ Wq64)
    b2 = Q * (Wk64.T @ bq64)
    a2t = np.ascontiguousarray(A2.T.astype(np.float32))          # [j, i]
    b2x = np.ascontiguousarray(
        np.repeat(b2.astype(np.float32)[:, None], B, axis=1))    # [D, B]
    qv0 = np.ascontiguousarray(
        query.astype(np.float64).mean(axis=1).T.astype(np.float32))  # [D, B]
    ident = np.eye(128, dtype=np.float32)

    dflat = documents.reshape(B, NL, D)
    in_maps = []
    for c in range(n_cores):
        shard = dflat[:, c * MC:(c + 1) * MC, :]          # [B, MC, D]
        d4 = shard.reshape(B, NTILE, 128, D).transpose(2, 0, 1, 3)
        in_maps.append({"docs": np.ascontiguousarray(d4).astype(np.float16),
                        "a2t": a2t, "b2x": b2x, "qv0": qv0, "ident": ident})
    return in_maps


def kernel(query, documents, Wq, bq, Wk, bk, max_steps):
    import time
    from concourse.bass_utils import run_bass_kernel_spmd

    steps = int(max_steps)
    if steps not in _cache:
        _cache[steps] = build(steps)
    nc = _cache[steps]

    in_maps = make_inputs(query, documents, Wq, bq, Wk, bk)
    last_exc = None
    for attempt in range(3):
        try:
            res = run_bass_kernel_spmd(nc, in_maps,
                                       core_ids=list(range(N_CORES)))
            break
        except Exception as e:  # noqa: BLE001
            last_exc = e
            time.sleep(15)
    else:
        raise last_exc
    outs = res.results[0]["outs"]                     # [steps*B, D], t-major
    return np.ascontiguousarray(
        outs.reshape(steps, B, D).transpose(1, 0, 2))  # (B, steps, D)
